# revision 26
# baseline (speedup 1.0000x reference)
"""Gaussian-mixture image renderer on 8 Trainium2 NeuronCores.

Math: for each image n (128 total) and pixel (r, w) of a 256x256 grid,
  y = clip( sum_k w_k e_k / max(sum_k e_k, 1e-7), 0, 1 ),
  e_k = exp(-0.5 * d^T (Sig Sig^T) d),  d = (gx - mux, gy - muy),
  gx = r/255, gy = w/255  (K = 4 gaussians per image).

The exponent is a quadratic in gy for each fixed image row r, so per
(row, n, k) it reduces to s = c0 + c1*tau + c2*tau^2 with tau = w the
integer pixel column index. On device this is ONE small f32r matmul per
row-pair: stationary = per-(row,n,k) coefficients (split hi/lo across 10
contraction rows for fp32-level accuracy), moving = a constant integer
feature tile [1, tau, a^2, a*b, b^2] (tau = 16a + b; all entries exact
in f32r's 11-bit mantissa).  ScalarE computes exp;  a second set of f32r
matmuls with zero-padded 128-column selector stationaries accumulates
g = sum_k e and yw = sum_k w_k e for four row-pairs into all 128 PSUM
partitions;  DVE does max(g,eps) -> approx-recip -> yw * (1/g);  GPSIMD
clips;  DMA writes image rows out.  Data-parallel over images: core i
computes images 16i..16i+15.
"""

import numpy as np
import sys

sys.path.insert(0, "/opt/trn_rl_repo")

H = 256
W = 256
K = 4
N_TOTAL = 128
N_CORES = 8
N_PER_CORE = N_TOTAL // N_CORES  # 16
TILES = H // 2  # 128 row-pair tiles per core
GROUPS = TILES // 8  # 16 groups of 8 tiles

_CACHE = {}


def _round11(x):
    """Round to 11 explicit mantissa bits (f32r's input rounding), RNE."""
    x = np.asarray(x, np.float64)
    mant, ex = np.frexp(x)
    return np.ldexp(np.round(mant * 4096.0) / 4096.0, ex)


def _host_precompute(params):
    """Per-core DRAM inputs from the full (128, 1, 28) params tensor."""
    p = np.asarray(params, np.float64).reshape(N_TOTAL, 7 * K)
    mux = p[:, 0:K]
    muy = p[:, K : 2 * K]
    wgt = p[:, 2 * K : 3 * K]
    S = p[:, 3 * K : 7 * K].reshape(N_TOTAL, K, 2, 2)
    s00 = S[:, :, 0, 0]
    s10 = S[:, :, 1, 0]
    s11 = S[:, :, 1, 1]
    a = s00 * s00
    b = s00 * s10
    c = s10 * s10 + s11 * s11
    # q = A gx^2 + B gy^2 + C gx gy + D gx + E gy + F
    A = a
    B = c
    C = 2.0 * b
    D = -(2.0 * a * mux + 2.0 * b * muy)
    E = -(2.0 * c * muy + 2.0 * b * mux)
    F = a * mux * mux + c * muy * muy + 2.0 * b * mux * muy

    lin = np.linspace(0.0, 1.0, W).astype(np.float32).astype(np.float64)
    gx = lin[:, None, None]  # (row r, n, k) broadcasting
    # s(tau) over a row: s = c0 + c1f*(tau/255) + c2f*(tau/255)^2
    c0 = -0.5 * (A[None] * gx * gx + D[None] * gx + F[None])  # (H, N, K)
    c1f = -0.5 * (C[None] * gx + E[None]) / 255.0
    c2f = np.broadcast_to(-0.5 * B[None] / (255.0 * 255.0), c0.shape)
    # features [1, tau, aa, ab, bb] with tau = 16a + b: tau^2 = 256aa + 32ab + bb
    co5 = np.stack([c0, c1f, 256.0 * c2f, 32.0 * c2f, c2f], axis=0)  # (5, H, N, K)
    co_hi = _round11(co5)
    co_lo = _round11(co5 - co_hi)

    tau = np.arange(W, dtype=np.float64)
    ta, tb = tau // 16.0, tau % 16.0
    feat5 = np.stack([np.ones(W), tau, ta * ta, ta * tb, tb * tb], axis=0)
    feat10 = np.concatenate([feat5, feat5], axis=0)  # (10, W)
    # block-diagonal moving tile for the 2-tiles-per-matmul mm1: rows 0-9
    # feed pixel cols 0-255 (tile 2m), rows 10-19 feed cols 256-511.
    ft = np.zeros((20, 2 * W), np.float64)
    ft[0:10, 0:W] = feat10
    ft[10:20, W : 2 * W] = feat10
    ft = ft.astype(np.float32)

    wr = _round11(wgt)  # (N, K)

    def lay(x):
        # (5, H, 16, 4) -> (5, TILES, 128): tile t covers rows
        # 16*(t//8) + t%8 + 8h for h in {0,1}; within-tile col j = 64h+4nl+k.
        x = x.reshape(5, GROUPS, 2, 8, N_PER_CORE, K)  # (5, G, h, j, nl, k)
        x = np.moveaxis(x, 2, 3)  # (5, G, j, h, nl, k)
        return x.reshape(5, TILES, 128)

    cores = []
    for ci in range(N_CORES):
        n0 = ci * N_PER_CORE
        hi = lay(co_hi[:, :, n0 : n0 + N_PER_CORE, :])
        lo = lay(co_lo[:, :, n0 : n0 + N_PER_CORE, :])
        # mm1 stationary m covers tiles 2m (rows 0-9) and 2m+1 (rows 10-19):
        # (20, 64, 128) -> (20, 64*128)
        co = np.zeros((20, TILES // 2, 128), np.float64)
        co[0:5] = hi[:, 0::2]
        co[5:10] = lo[:, 0::2]
        co[10:15] = hi[:, 1::2]
        co[15:20] = lo[:, 1::2]
        co = co.reshape(20, (TILES // 2) * 128).astype(np.float32)

        # sel: (128, 1024): 8 zero-padded stationaries of 128 cols each.
        # Stationary u (pair u = tiles 2u,2u+1 of an 8-tile group) places its
        # outputs at partition 8*nl + B with B = u+4h; B indexes row-pair
        # 16g+2B+{0,1}, so one full-tile DMA (partition-major = image-major)
        # stores the whole group. Blocks 0-3 = G selectors (ones), 4-7 = W.
        sel = np.zeros((128, 1024), np.float32)
        for u in range(4):
            for hh in range(2):
                for nl in range(N_PER_CORE):
                    col = 8 * nl + (u + 4 * hh)
                    for kk in range(K):
                        row = 64 * hh + 4 * nl + kk
                        sel[row, 128 * u + col] = 1.0
                        sel[row, 512 + 128 * u + col] = wr[n0 + nl, kk]
        cores.append({"co": co, "ft": ft, "sel": sel})
    return cores


def _build_program():
    import concourse.bacc as bacc
    import concourse.tile as tile
    from concourse import mybir
    from contextlib import ExitStack

    F32 = mybir.dt.float32
    F32R = mybir.dt.float32r
    nc = bacc.Bacc()
    co_d = nc.declare_dram_parameter(
        "co", [20, (TILES // 2) * 128], F32R, isOutput=False
    )
    ft_d = nc.declare_dram_parameter("ft", [20, 2 * W], F32R, isOutput=False)
    sel_d = nc.declare_dram_parameter("sel", [128, 1024], F32R, isOutput=False)
    out_d = nc.declare_dram_parameter("out", [N_PER_CORE, H * W], F32, isOutput=True)
    # row R = 16*g + 2*B + c: view (n, g, B, c, x)
    outv = out_d[:, :].rearrange("n (G B c x) -> n G B c x", B=8, c=2, x=W)

    with ExitStack() as ctx:
        tc = ctx.enter_context(tile.TileContext(nc))
        singles = ctx.enter_context(tc.tile_pool(name="singles", bufs=1))
        psum = ctx.enter_context(tc.tile_pool(name="psum", bufs=2, space="PSUM"))
        epool = ctx.enter_context(tc.tile_pool(name="epool", bufs=8))
        npool = ctx.enter_context(tc.tile_pool(name="npool", bufs=6))
        ypool = ctx.enter_context(tc.tile_pool(name="ypool", bufs=16))

        CO = singles.tile([20, (TILES // 2) * 128], F32R, tag="CO")
        FT = singles.tile([20, 2 * W], F32R, tag="FT")
        SEL = singles.tile([128, 1024], F32R, tag="SEL")
        # group 0's coefficients first so mm1 starts ASAP; bulk follows
        nc.sync.dma_start(out=CO[:, 0:512], in_=co_d[:, 0:512])
        nc.sync.dma_start(out=FT, in_=ft_d[:, :])
        nc.sync.dma_start(out=SEL, in_=sel_d[:, :])
        nc.sync.dma_start(out=CO[:, 512:], in_=co_d[:, 512:])
        # pre-warm the Exp table set while input DMAs are in flight
        warm = npool.tile([128, 8], F32, tag="warm")
        nc.vector.memset(warm, 0.0)
        nc.scalar.activation(
            warm, warm, mybir.ActivationFunctionType.Exp, 0.0, 1.0, 0.0
        )

        # Software pipeline: issue group g's mm1+exp, then group g-1's
        # mm2+normalize+store, so PE runs mm2(g-1) while ACT computes exp(g).
        e_prev = None
        for g in range(GROUPS + 1):
            e_cur = []
            if g < GROUPS:
                for bb in range(2):
                    ps1 = psum.tile([128, 1024], F32, tag="ps1")
                    for i in range(2):
                        m = 4 * g + 2 * bb + i  # covers tiles 2m, 2m+1
                        nc.tensor.matmul(
                            ps1[:, 512 * i : 512 * (i + 1)],
                            CO[:, 128 * m : 128 * (m + 1)],
                            FT,
                            start=True,
                            stop=True,
                        )
                    et = epool.tile([128, 1024], F32R, tag="E")
                    nc.scalar.activation(
                        et, ps1, mybir.ActivationFunctionType.Exp, 0.0, 1.0, 0.0
                    )
                    e_cur.append(et)
            if e_prev is not None:
                gp = g - 1
                psG = psum.tile([128, 512], F32, tag="psG")
                psW = psum.tile([128, 512], F32, tag="psW")
                for u in range(4):
                    src = e_prev[u // 2][:, 512 * (u % 2) : 512 * (u % 2) + 512]
                    nc.tensor.matmul(
                        psG,
                        SEL[:, 128 * u : 128 * (u + 1)],
                        src,
                        start=(u == 0),
                        stop=(u == 3),
                    )
                    nc.tensor.matmul(
                        psW,
                        SEL[:, 512 + 128 * u : 512 + 128 * (u + 1)],
                        src,
                        start=(u == 0),
                        stop=(u == 3),
                    )
                gs = npool.tile([128, 512], F32, tag="gs")
                nc.vector.tensor_scalar_max(gs, psG, 1e-7)
                rg = npool.tile([128, 512], F32, tag="rg")
                nc.vector.reciprocal_approx_fast(out=rg, in_=gs)
                yv = npool.tile([128, 512], F32, tag="yv")
                nc.vector.tensor_mul(yv, rg, psW)
                yc = ypool.tile([128, 512], F32, tag="yc")
                nc.gpsimd.tensor_scalar(
                    yc,
                    yv,
                    0.0,
                    1.0,
                    op0=mybir.AluOpType.max,
                    op1=mybir.AluOpType.min,
                )
                # one DMA per group: src partitions 8nl+B (image-major) ->
                # dst rows 16gp+2B+c per image nl; (c,x) contiguous 2KB runs
                nc.sync.dma_start(out=outv[:, gp], in_=yc)
            e_prev = e_cur

    nc.finalize()
    return nc


def kernel(height, width, params):
    assert int(height) == H and int(width) == W
    from concourse.bass_utils import run_bass_kernel_spmd

    if "nc" not in _CACHE:
        _CACHE["nc"] = _build_program()
    nc = _CACHE["nc"]

    cores = _host_precompute(params)
    in_maps = [dict(c) for c in cores]
    res = run_bass_kernel_spmd(nc, in_maps, core_ids=list(range(N_CORES)))
    out = np.concatenate(
        [res.results[ci]["out"] for ci in range(N_CORES)], axis=0
    )
    return out.reshape(N_TOTAL, 1, H, W).astype(np.float32)
